# revision 1
# baseline (speedup 1.0000x reference)
"""C3D-style circulant-block 3D CNN forward pass on 8 Trainium2 NeuronCores.

Sharding: data-parallel over batch (8 samples -> 8 cores). Training-mode
BatchNorm batch statistics are combined across cores with a tiny per-layer
f32 AllReduce of (mean, E[x^2]) per channel.

Device kernel per core (per sample):
  conv1 via host-side im2col (K=81 = ci*kd*kh*kw) -> single matmul stream
  conv2..conv5b as shift-and-accumulate implicit GEMM: input channels on
  partitions (K-chunks of 128), 27 taps accumulated in PSUM, strided
  interior access patterns on padded SBUF buffers. conv2 packs (ci, 2 kd
  planes) into K=128 + a K=64 remainder.
  Per conv tile: ACT copies PSUM->bf16, DVE bn_stats accumulates BN stats,
  DVE tensor_max performs maxpool (pool applied to raw values before the
  BN affine; valid because the BN scale g*rsqrt(var+eps) is positive).
  After each conv: bn_aggr -> AllReduce(mean, E2) -> scale/shift -> fused
  BN+ReLU via one ACT op writing the next layer's padded input.
  Tail: special-padded pool5 -> global mean (folded /16 into FC weights)
  -> FC matmul -> logits.
"""

import numpy as np
import ml_dtypes

import concourse.bass as bass
import concourse.mybir as mybir
import concourse.tile as tile
from concourse import bacc
from concourse.bass_utils import run_bass_kernel_spmd

F32 = mybir.dt.float32
BF16 = mybir.dt.bfloat16
NPBF16 = ml_dtypes.bfloat16
RELU = mybir.ActivationFunctionType.Relu
COPY = mybir.ActivationFunctionType.Copy
SQRT = mybir.ActivationFunctionType.Sqrt
IDENT = mybir.ActivationFunctionType.Identity
ADD = mybir.AluOpType.add
EPS = 1e-5
N_CORES = 8

# name, Cin, Cout, D, H, W, R(rows/tile), zpair, pooled
GEN_LAYERS = [
    ("3a", 128, 256, 8, 28, 28, 14, False, False),
    ("3b", 256, 256, 8, 28, 28, 14, False, True),
    ("4a", 256, 512, 4, 14, 14, 14, True, False),
    ("4b", 512, 512, 4, 14, 14, 14, True, True),
    ("5a", 512, 512, 2, 7, 7, 7, True, False),
    ("5b", 512, 512, 2, 7, 7, 7, True, None),  # None -> raw stage (special pool)
]

TAPS = [(kd, kh, kw) for kd in range(3) for kh in range(3) for kw in range(3)]


def circ_expand_np(c):
    c = np.asarray(c, np.float32)
    P, Q, b = c.shape[0], c.shape[1], c.shape[2]
    r = np.arange(b)
    idx = (r[:, None] - r[None, :]) % b
    w = c[:, :, idx]  # (P, Q, b, b, k, k, k)
    w = np.transpose(w, (0, 2, 1, 3, 4, 5, 6))
    return w.reshape(P * b, Q * b, *c.shape[3:])


def pack_w_generic(wd, Kch, Mch):
    # wd (Co, Ci, 3,3,3) -> [Mch, 128ci, Kch, 27, 128co] bf16
    Co, Ci = wd.shape[0], wd.shape[1]
    wt = wd.transpose(1, 2, 3, 4, 0)  # (Ci, kd,kh,kw, Co)
    wt = wt.reshape(Kch, 128, 27, Mch, 128)
    wt = wt.transpose(3, 1, 0, 2, 4)  # (m, ci, c, t, co)
    return np.ascontiguousarray(wt, dtype=NPBF16)


def host_prep(inputs):
    g = {k: np.asarray(v, np.float32) for k, v in inputs.items()}
    shared = {}
    # conv1
    w1 = g["conv1_w"]  # (64, 3, 3,3,3)
    shared["w1"] = np.ascontiguousarray(
        w1.transpose(1, 2, 3, 4, 0).reshape(81, 64), dtype=NPBF16)
    # conv2 (kd-packed)
    w2 = circ_expand_np(g["c2"])  # (128, 64, 3,3,3)
    w2t = w2.transpose(2, 1, 3, 4, 0)  # (kd, ci, kh, kw, co)
    shared["w2a"] = np.ascontiguousarray(
        w2t[0:2].reshape(128, 9, 128), dtype=NPBF16)
    shared["w2b"] = np.ascontiguousarray(
        w2t[2].reshape(64, 9, 128), dtype=NPBF16)
    # generic layers
    for (name, Cin, Cout, *_rest) in GEN_LAYERS:
        wd = circ_expand_np(g[f"c{name}"])
        shared[f"w{name}"] = pack_w_generic(wd, Cin // 128, Cout // 128)
    # bn params
    def pk(v, parts):
        v = np.asarray(v, np.float32)
        mch = v.size // parts
        return np.ascontiguousarray(v.reshape(mch, parts).T)
    shared["gn1"] = pk(g["g1"], 64)
    shared["bn1"] = pk(g["b1"], 64)
    for name, c in [("2", 128), ("3a", 256), ("3b", 256), ("4a", 512),
                    ("4b", 512), ("5a", 512), ("5b", 512)]:
        shared[f"gn{name}"] = pk(g[f"g{name}"], 128)
        shared[f"bn{name}"] = pk(g[f"b{name}"], 128)
        assert np.all(g[f"g{name}"] >= 0), "pool/BN commute needs g >= 0"
    assert np.all(g["g1"] >= 0)
    # fc (fold /16 global-mean into weights)
    fcw = (g["fc_w"].T / 16.0)  # (512, 101)
    shared["fcw"] = np.ascontiguousarray(
        fcw.reshape(4, 128, 101).transpose(1, 0, 2), dtype=NPBF16)
    shared["fcb"] = np.ascontiguousarray(g["fc_b"].reshape(101, 1))
    # per-core conv1 im2col
    x = g["x"]  # (8, 3, 16, 112, 112)
    x1_list = []
    for i in range(x.shape[0]):
        xp = np.zeros((3, 18, 114, 114), np.float32)
        xp[:, 1:17, 1:113, 1:113] = x[i]
        sw = np.lib.stride_tricks.sliding_window_view(xp, (3, 3, 3), axis=(1, 2, 3))
        b1 = sw.transpose(0, 4, 5, 6, 1, 2, 3).reshape(81, 16, 12544)
        x1_list.append(np.ascontiguousarray(b1, dtype=NPBF16))
    return shared, x1_list


def build_bass(n_cores, fake_cc=False):
    nc = bacc.Bacc("TRN2", target_bir_lowering=False, debug=False,
                   num_devices=n_cores)
    rg = [list(range(n_cores))]

    din = {}
    din["x1"] = nc.dram_tensor("x1", [81, 16, 12544], BF16, kind="ExternalInput")
    din["w1"] = nc.dram_tensor("w1", [81, 64], BF16, kind="ExternalInput")
    din["w2a"] = nc.dram_tensor("w2a", [128, 9, 128], BF16, kind="ExternalInput")
    din["w2b"] = nc.dram_tensor("w2b", [64, 9, 128], BF16, kind="ExternalInput")
    for (name, Cin, Cout, *_r) in GEN_LAYERS:
        din[f"w{name}"] = nc.dram_tensor(
            f"w{name}", [Cout // 128, 128, Cin // 128, 27, 128], BF16,
            kind="ExternalInput")
    din["gn1"] = nc.dram_tensor("gn1", [64, 1], F32, kind="ExternalInput")
    din["bn1"] = nc.dram_tensor("bn1", [64, 1], F32, kind="ExternalInput")
    for name, c in [("2", 128), ("3a", 256), ("3b", 256), ("4a", 512),
                    ("4b", 512), ("5a", 512), ("5b", 512)]:
        mch = c // 128
        din[f"gn{name}"] = nc.dram_tensor(f"gn{name}", [128, mch], F32,
                                          kind="ExternalInput")
        din[f"bn{name}"] = nc.dram_tensor(f"bn{name}", [128, mch], F32,
                                          kind="ExternalInput")
    din["fcw"] = nc.dram_tensor("fcw", [128, 4, 101], BF16, kind="ExternalInput")
    din["fcb"] = nc.dram_tensor("fcb", [101, 1], F32, kind="ExternalInput")
    logits = nc.dram_tensor("logits", [101, 1], F32, kind="ExternalOutput")
    stats_out = {}
    for name, parts, mch in [("1", 64, 1), ("2", 128, 1), ("3a", 128, 2),
                             ("3b", 128, 2), ("4a", 128, 4), ("4b", 128, 4),
                             ("5a", 128, 4), ("5b", 128, 4)]:
        stats_out[name] = nc.dram_tensor(f"st{name}", [parts, mch * 2], F32,
                                         kind="ExternalOutput")

    with tile.TileContext(nc) as tc:
        build_graph(tc, din, logits, stats_out, rg, fake_cc)
    nc.compile()
    return nc


def build_graph(tc, din, logits, stats_out, rg, fake_cc=False):
    nc = tc.nc
    import contextlib
    ctx = contextlib.ExitStack()
    with ctx:
        singles = ctx.enter_context(tc.tile_pool(name="singles", bufs=1))
        small = ctx.enter_context(tc.tile_pool(name="small", bufs=2))
        statsp = ctx.enter_context(tc.tile_pool(name="statsp", bufs=2))
        psum = ctx.enter_context(tc.tile_pool(name="psum", bufs=3, space="PSUM"))
        psfc = ctx.enter_context(tc.tile_pool(name="psfc", bufs=1, space="PSUM"))
        ybfp = ctx.enter_context(tc.tile_pool(name="ybfp", bufs=4))
        pwp = ctx.enter_context(tc.tile_pool(name="pwp", bufs=4))
        stagep = ctx.enter_context(tc.tile_pool(name="stagep", bufs=1))
        ypoolp = ctx.enter_context(tc.tile_pool(name="ypoolp", bufs=1))
        arena = ctx.enter_context(tc.tile_pool(name="arena", bufs=1))
        dram = ctx.enter_context(tc.tile_pool(name="dram", bufs=1, space="DRAM"))

        eps_t = singles.tile([128, 1], F32, tag="eps")
        nc.vector.memset(eps_t[:], EPS)

        # persistent small params
        params = {}
        for name, parts in [("1", 64), ("2", 128), ("3a", 128), ("3b", 128),
                            ("4a", 128), ("4b", 128), ("5a", 128), ("5b", 128)]:
            mch = din[f"gn{name}"].shape[1]
            gt = singles.tile([parts, mch], F32, tag=f"g{name}")
            bt = singles.tile([parts, mch], F32, tag=f"b{name}")
            nc.sync.dma_start(gt[:], din[f"gn{name}"][:])
            nc.sync.dma_start(bt[:], din[f"bn{name}"][:])
            params[name] = (gt, bt)

        w1_sb = singles.tile([81, 64], BF16, tag="w1")
        nc.sync.dma_start(w1_sb[:], din["w1"][:])
        w2a_sb = singles.tile([128, 9, 128], BF16, tag="w2a")
        nc.sync.dma_start(w2a_sb[:], din["w2a"][:])
        w2b_sb = singles.tile([64, 9, 128], BF16, tag="w2b")
        nc.sync.dma_start(w2b_sb[:], din["w2b"][:])
        fcw_sb = singles.tile([128, 4, 101], BF16, tag="fcw")
        nc.sync.dma_start(fcw_sb[:], din["fcw"][:])
        fcb_sb = singles.tile([101, 1], F32, tag="fcb")
        nc.sync.dma_start(fcb_sb[:], din["fcb"][:])

        y1_dram = dram.tile([64, 16, 3136], BF16, tag="y1d")

        def bn_reduce(name, stats_t, parts, mch):
            """stats_t [parts, mch, T, 6] -> (s, t) [parts, mch] f32."""
            mv = small.tile([parts, mch, 2], F32, tag="mv")
            for m in range(mch):
                nc.vector.bn_aggr(mv[:, m], stats_t[:, m])
            cc = small.tile([parts, mch, 2], F32, tag="cc")
            sq = small.tile([parts, mch], F32, tag="sq")
            nc.vector.tensor_mul(sq[:], mv[:, :, 0], mv[:, :, 0])
            nc.vector.tensor_add(cc[:, :, 1], mv[:, :, 1], sq[:])
            nc.vector.tensor_copy(cc[:, :, 0], mv[:, :, 0])
            ccin = dram.tile([parts, mch * 2], F32, tag=f"ccin{name}")
            ccout = dram.tile([parts, mch * 2], F32, tag=f"ccout{name}",
                              addr_space="Shared")
            nc.sync.dma_start(ccin[:], cc[:].rearrange("p m two -> p (m two)"))
            if fake_cc:
                nc.sync.dma_start(ccout[:], ccin[:])
            else:
                nc.gpsimd.collective_compute(
                    "AllReduce", ADD, replica_groups=rg,
                    ins=[ccin.opt()], outs=[ccout.opt()])
            ar = small.tile([parts, mch, 2], F32, tag="ar")
            nc.sync.dma_start(ar[:].rearrange("p m two -> p (m two)"), ccout[:])
            nc.sync.dma_start(stats_out[name][:],
                              ar[:].rearrange("p m two -> p (m two)"))
            inv_n = 1.0 / len(rg[0])
            mg = small.tile([parts, mch], F32, tag="mg")
            e2 = small.tile([parts, mch], F32, tag="e2")
            nc.vector.tensor_scalar_mul(mg[:], ar[:, :, 0], inv_n)
            nc.vector.tensor_scalar_mul(e2[:], ar[:, :, 1], inv_n)
            sq2 = small.tile([parts, mch], F32, tag="sq2")
            nc.vector.tensor_mul(sq2[:], mg[:], mg[:])
            varg = small.tile([parts, mch], F32, tag="varg")
            nc.vector.tensor_sub(varg[:], e2[:], sq2[:])
            sd = small.tile([parts, mch], F32, tag="sd")
            nc.scalar.activation(sd[:], varg[:], SQRT, bias=eps_t[:parts])
            inv = small.tile([parts, mch], F32, tag="inv")
            nc.vector.reciprocal(inv[:], sd[:])
            gt, bt = params[name]
            s_t = small.tile([parts, mch], F32, tag="s_t")
            nc.vector.tensor_mul(s_t[:], inv[:], gt[:])
            tmn = small.tile([parts, mch], F32, tag="tmn")
            nc.vector.tensor_mul(tmn[:], mg[:], s_t[:])
            t_t = small.tile([parts, mch], F32, tag="t_t")
            nc.vector.tensor_sub(t_t[:], bt[:], tmn[:])
            return s_t, t_t

        def zero_borders(P, mch, Dp, Hp, Wp):
            for c in range(mch):
                nc.vector.memset(P[:, c, 0], 0.0)
                nc.vector.memset(P[:, c, Dp - 1], 0.0)
                nc.vector.memset(P[:, c, 1:Dp - 1, 0, :], 0.0)
                nc.vector.memset(P[:, c, 1:Dp - 1, Hp - 1, :], 0.0)
                nc.vector.memset(P[:, c, 1:Dp - 1, 1:Hp - 1, 0:1], 0.0)
                nc.vector.memset(P[:, c, 1:Dp - 1, 1:Hp - 1, Wp - 1:Wp], 0.0)

        # ---------------- conv1 ----------------
        stats1 = statsp.tile([64, 1, 448, 6], F32, tag="stats")
        with tc.tile_pool(name="x1p", bufs=3) as x1p, \
             tc.tile_pool(name="zplp", bufs=2) as zplp, \
             nc.named_scope("conv1"):
            for z in range(16):
                zplane = zplp.tile([64, 56, 56], BF16, tag="zpl")
                for half in range(2):
                    slab = x1p.tile([81, 6272], BF16, tag="slab")
                    nc.sync.dma_start(
                        slab[:], din["x1"][:, z, half * 6272:(half + 1) * 6272])
                    for t2 in range(7):
                        pst = psum.tile([64, 2, 512], F32, tag="ps", name="ps")
                        for j in range(2):
                            t = 2 * t2 + j
                            nc.tensor.matmul(pst[:, j, :448], w1_sb[:],
                                             slab[:, t * 448:(t + 1) * 448],
                                             start=True, stop=True)
                        ybft = ybfp.tile([64, 1024], BF16, tag="ybf",
                                         name="ybf")
                        ybf = ybft[:, :896]
                        nc.scalar.activation(
                            ybf.rearrange("p (g n) -> p g n", g=2),
                            pst[:, :, :448], COPY)
                        ti = z * 28 + half * 14 + 2 * t2
                        nc.vector.bn_stats(stats1[:, 0, ti], ybft[:, 0:448])
                        nc.vector.bn_stats(stats1[:, 0, ti + 1],
                                           ybft[:, 448:896])
                        v = ybf.rearrange("p (a b) -> p a b", a=8)
                        pw = pwp.tile([64, 8, 56], BF16, tag="pw")
                        nc.vector.tensor_max(pw[:], v[:, :, 0::2], v[:, :, 1::2])
                        ro = half * 28 + 4 * t2
                        nc.vector.tensor_max(zplane[:, ro:ro + 4, :],
                                             pw[:, 0::2, :], pw[:, 1::2, :])
                nc.sync.dma_start(y1_dram[:, z, :],
                                  zplane[:].rearrange("p a b -> p (a b)"))
        with nc.named_scope("ar1"):
            s1, t1 = bn_reduce("1", stats1, 64, 1)

        # ---------------- conv2 ----------------
        stats2 = statsp.tile([128, 1, 128, 6], F32, tag="stats")
        S2 = stagep.tile([128, 1, 16, 28, 28], BF16, tag="stage")
        with tc.tile_pool(name="plp", bufs=3) as plp, \
             tc.tile_pool(name="b2p", bufs=2) as b2p, \
             tc.tile_pool(name="c2p", bufs=2) as c2p, \
             nc.named_scope("conv2"):

            def build_plane(dst64, pidx):
                if pidx == 0 or pidx == 17:
                    nc.vector.memset(dst64[:], 0.0)
                    return
                pl = plp.tile([64, 3136], BF16, tag="pl")
                nc.sync.dma_start(pl[:], y1_dram[:, pidx - 1, :])
                nc.vector.memset(dst64[:, 0, :], 0.0)
                nc.vector.memset(dst64[:, 57, :], 0.0)
                nc.vector.memset(dst64[:, 1:57, 0:1], 0.0)
                nc.vector.memset(dst64[:, 1:57, 57:58], 0.0)
                nc.scalar.activation(
                    dst64[:, 1:57, 1:57],
                    pl[:].rearrange("p (a b) -> p a b", a=56),
                    RELU, bias=t1[:, 0:1], scale=s1[:, 0:1])

            for z in range(16):
                B2 = b2p.tile([128, 58, 58], BF16, tag="b2")
                build_plane(B2[0:64], z)
                build_plane(B2[64:128], z + 1)
                C2 = c2p.tile([64, 58, 58], BF16, tag="c2")
                build_plane(C2[:], z + 2)
                for p2 in range(4):
                    pst = psum.tile([128, 2, 512], F32, tag="ps", name="ps")
                    for k9 in range(9):
                        kh, kw = k9 // 3, k9 % 3
                        for j in range(2):
                            y0 = 14 * p2 + 7 * j + kh
                            nc.tensor.matmul(pst[:, j, :392], w2a_sb[:, k9, :],
                                             B2[:, y0:y0 + 7, kw:kw + 56],
                                             start=(k9 == 0), stop=False)
                    for k9 in range(9):
                        kh, kw = k9 // 3, k9 % 3
                        for j in range(2):
                            y0 = 14 * p2 + 7 * j + kh
                            nc.tensor.matmul(pst[:, j, :392], w2b_sb[:, k9, :],
                                             C2[:, y0:y0 + 7, kw:kw + 56],
                                             start=False, stop=(k9 == 8))
                    ybft = ybfp.tile([128, 1024], BF16, tag="ybf", name="ybf")
                    ybf = ybft[:, :784]
                    nc.vector.tensor_copy(
                        ybf.rearrange("p (g n) -> p g n", g=2),
                        pst[:, :, :392])
                    nc.vector.bn_stats(stats2[:, 0, z * 8 + 2 * p2],
                                       ybft[:, 0:392])
                    nc.vector.bn_stats(stats2[:, 0, z * 8 + 2 * p2 + 1],
                                       ybft[:, 392:784])
                    v = ybf.rearrange("p (a b) -> p a b", a=14)
                    pw = pwp.tile([128, 14, 28], BF16, tag="pw")
                    nc.vector.tensor_max(pw[:], v[:, :, 0::2], v[:, :, 1::2])
                    nc.vector.tensor_max(S2[:, 0, z, 7 * p2:7 * p2 + 7, :],
                                         pw[:, 0::2, :], pw[:, 1::2, :])
        with nc.named_scope("ar2"):
            s2, t2 = bn_reduce("2", stats2, 128, 1)
        Y2p = ypoolp.tile([128, 1, 8, 28, 28], BF16, tag="ypool")
        nc.vector.tensor_max(Y2p[:, 0], S2[:, 0, 0::2], S2[:, 0, 1::2])
        P3in = arena.tile([128, 1, 10, 30, 30], BF16, tag="pin")
        zero_borders(P3in, 1, 10, 30, 30)
        nc.scalar.activation(P3in[:, 0, 1:9, 1:29, 1:29], Y2p[:, 0], RELU,
                             bias=t2[:, 0:1], scale=s2[:, 0:1])

        # ---------------- generic conv layers ----------------
        with tc.tile_pool(name="wp", bufs=3) as wp:
            Pin = P3in
            for (name, Cin, Cout, D, H, W, R, zpair, pooled) in GEN_LAYERS:
                Kch, Mch = Cin // 128, Cout // 128
                ntz = D // 2 if zpair else D
                zcnt = 2 if zpair else 1
                ytiles = H // R
                N = zcnt * R * W
                T = ntz * ytiles
                H2, W2, D2 = H // 2, W // 2, D // 2
                stats_t = statsp.tile([128, Mch, T, 6], F32, tag="stats")
                if pooled is False or pooled is None:
                    stage = stagep.tile([128, Mch, D, H, W], BF16, tag="stage")
                else:
                    stage = stagep.tile([128, Mch, D, H2, W2], BF16, tag="stage")
                stage_flat = stage[:].rearrange("p m d h w -> p (m d h w)")
                scope = ctx2 = nc.named_scope(f"conv{name}")
                ctx2.__enter__()
                tiles = [(2 * tz if zpair else tz, ty * R)
                         for tz in range(ntz) for ty in range(ytiles)]
                groups = [tiles[i:i + 2] for i in range(0, len(tiles), 2)]
                for m in range(Mch):
                    wm = wp.tile([128, Kch, 27, 128], BF16, tag="w")
                    nc.sync.dma_start(wm[:, :Kch], din[f"w{name}"][m])
                    ti = 0
                    for grp in groups:
                        G = len(grp)
                        pst = psum.tile([128, 2, 512], F32, tag="ps",
                                        name="ps")
                        nmm = Kch * 27
                        i = 0
                        for c in range(Kch):
                            for (kd, kh, kw) in TAPS:
                                for j, (z0, y0) in enumerate(grp):
                                    rhs = Pin[:, c, z0 + kd:z0 + kd + zcnt,
                                              y0 + kh:y0 + kh + R,
                                              kw:kw + W]
                                    nc.tensor.matmul(
                                        pst[:, j, :N],
                                        wm[:, c, kd * 9 + kh * 3 + kw, :],
                                        rhs, start=(i == 0), stop=(i == nmm - 1))
                                i += 1
                        z0, y0 = grp[0]
                        if pooled is False or pooled is None:
                            off = (m * D + z0) * H * W + y0 * W
                            dst = stage_flat[:, off:off + G * N]
                            nc.vector.tensor_copy(
                                dst.rearrange("p (g n) -> p g n", g=G),
                                pst[:, :G, :N])
                            for j in range(G):
                                nc.vector.bn_stats(
                                    stats_t[:, m, ti + j],
                                    stage_flat[:, off + j * N:off + (j + 1) * N])
                        else:
                            ybft = ybfp.tile([128, 1024], BF16,
                                             tag="ybf", name="ybf")
                            ybf = ybft[:, :G * N]
                            nc.vector.tensor_copy(
                                ybf.rearrange("p (g n) -> p g n", g=G),
                                pst[:, :G, :N])
                            for j in range(G):
                                nc.vector.bn_stats(
                                    stats_t[:, m, ti + j],
                                    ybft[:, j * N:(j + 1) * N])
                            nz = G * zcnt if zpair else 1
                            nr = R if zpair else G * R
                            v = ybf.rearrange("p (z y x) -> p z y x",
                                              z=nz, y=nr)
                            pw = pwp.tile([128, nz, nr, W2], BF16,
                                          tag="pw2", name="pw")
                            nc.vector.tensor_max(pw[:], v[:, :, :, 0::2],
                                                 v[:, :, :, 1::2])
                            nc.vector.tensor_max(
                                stage[:, m, z0:z0 + nz,
                                      y0 // 2:y0 // 2 + nr // 2, :],
                                pw[:, :, 0::2, :], pw[:, :, 1::2, :])
                        ti += G
                ctx2.__exit__(None, None, None)
                with nc.named_scope(f"ar{name}"):
                    s_t, t_t = bn_reduce(name, stats_t, 128, Mch)

                if name == "5b":
                    # pool5: window (2,2,2) stride 2, pad (0,1,1)
                    pd = small.tile([128, 4, 7, 7], BF16, tag="pd5")
                    nc.vector.tensor_max(pd[:], stage[:, :, 0], stage[:, :, 1])
                    pw5 = small.tile([128, 4, 7, 4], BF16, tag="pw5")
                    nc.vector.tensor_copy(pw5[:, :, :, 0:1], pd[:, :, :, 0:1])
                    nc.vector.tensor_max(pw5[:, :, :, 1:4],
                                         pd[:, :, :, 1::2], pd[:, :, :, 2::2])
                    ph5 = small.tile([128, 4, 4, 4], BF16, tag="ph5")
                    nc.vector.tensor_copy(ph5[:, :, 0:1, :], pw5[:, :, 0:1, :])
                    nc.vector.tensor_max(ph5[:, :, 1:4, :],
                                         pw5[:, :, 1::2, :], pw5[:, :, 2::2, :])
                    # BN+ReLU -> Z, then global mean (1/16 folded into fcw)
                    Z = small.tile([128, 4, 16], BF16, tag="z5")
                    for m in range(4):
                        nc.scalar.activation(
                            Z[:, m, :],
                            ph5[:, m].rearrange("p a b -> p (a b)"),
                            RELU, bias=t_t[:, m:m + 1], scale=s_t[:, m:m + 1])
                    feat = small.tile([128, 4], F32, tag="feat")
                    nc.vector.tensor_reduce(feat[:], Z[:],
                                            axis=mybir.AxisListType.X, op=ADD)
                    fcin = small.tile([128, 4], BF16, tag="fcin")
                    nc.vector.tensor_copy(fcin[:], feat[:])
                    psf = psfc.tile([101, 1], F32, tag="psfc")
                    for c in range(4):
                        nc.tensor.matmul(psf[:], fcw_sb[:, c, :],
                                         fcin[:, c:c + 1],
                                         start=(c == 0), stop=(c == 3))
                    out_sb = small.tile([101, 1], F32, tag="outsb")
                    nc.scalar.activation(out_sb[:], psf[:], IDENT,
                                         bias=fcb_sb[:])
                    nc.sync.dma_start(logits[:], out_sb[:])
                    break

                # D-pool (if pooled) then BN+ReLU apply into next padded input
                if pooled:
                    src = ypoolp.tile([128, Mch, D2, H2, W2], BF16, tag="ypool")
                    for m in range(Mch):
                        nc.vector.tensor_max(src[:, m], stage[:, m, 0::2],
                                             stage[:, m, 1::2])
                    nD, nH, nW = D2, H2, W2
                else:
                    src = stage
                    nD, nH, nW = D, H, W
                Pnext = arena.tile([128, Mch, nD + 2, nH + 2, nW + 2], BF16,
                                   tag="pin")
                zero_borders(Pnext, Mch, nD + 2, nH + 2, nW + 2)
                for m in range(Mch):
                    nc.scalar.activation(
                        Pnext[:, m, 1:1 + nD, 1:1 + nH, 1:1 + nW],
                        src[:, m], RELU,
                        bias=t_t[:, m:m + 1], scale=s_t[:, m:m + 1])
                Pin = Pnext


_STATE = {}


def _get_nc(n_cores=N_CORES):
    key = f"nc{n_cores}"
    if key not in _STATE:
        _STATE[key] = build_bass(n_cores)
    return _STATE[key]


def kernel(**inputs):
    nc = _get_nc()
    shared, x1_list = host_prep(inputs)
    in_maps = []
    for i in range(N_CORES):
        m = dict(shared)
        m["x1"] = x1_list[i]
        in_maps.append(m)
    res = run_bass_kernel_spmd(nc, in_maps, core_ids=list(range(N_CORES)))
    out = np.stack([res.results[i]["logits"].reshape(101)
                    for i in range(N_CORES)]).astype(np.float32)
    return out



# revision 10
# speedup vs baseline: 1.1830x; 1.1830x over previous
"""C3D-style circulant-block 3D CNN forward pass on 8 Trainium2 NeuronCores.

Sharding: data-parallel over batch (8 samples -> 8 cores). Training-mode
BatchNorm batch statistics are combined across cores with small per-layer
(or per-M-chunk) f32 AllReduces of (mean, E[x^2]) per channel.

Device kernel per core (per sample):
  conv1 via host-side im2col (K=81) -> matmul stream; the two row-tiles of
  each step land in PSUM partitions 0-63 / 64-127 (PE column tiling), so
  BN-stats and maxpool run on all 128 partitions straight off PSUM.
  conv2 packs (ci, 2 kd planes) into K=128 + kd=2 remainder packed as
  (ci, 2 kh shifts) K=128 + K=64 leftover (15 instead of 18 matmuls/tile).
  conv3a..5b as shift-and-accumulate implicit GEMM, 27 taps accumulated in
  PSUM, strided interior access on padded SBUF buffers.
  BN stats AllReduces are pipelined: per-M-chunk for 3a/3b/4a/4b (hidden
  under the next chunk's matmuls), spatially split for conv1/conv2, single
  for 5a/5b. Pool/stats read PSUM directly; psum->SBUF copies ride the
  Scalar engine. BN+ReLU apply fused into one ACT op per chunk writing the
  next layer's padded input; maxpool on raw values (valid: g >= 0).
  Tail: special-padded pool5 -> global mean (1/16 folded into FC weights)
  -> FC matmul -> logits.
"""

import numpy as np
import ml_dtypes

import concourse.bass as bass
import concourse.mybir as mybir
import concourse.tile as tile
from concourse import bacc
from concourse.bass_utils import run_bass_kernel_spmd

F32 = mybir.dt.float32
BF16 = mybir.dt.bfloat16
NPBF16 = ml_dtypes.bfloat16
RELU = mybir.ActivationFunctionType.Relu
COPY = mybir.ActivationFunctionType.Copy
SQRT = mybir.ActivationFunctionType.Sqrt
IDENT = mybir.ActivationFunctionType.Identity
ADD = mybir.AluOpType.add
EPS = 1e-5
N_CORES = 8

# name, Cin, Cout, D, H, W, R(rows/tile), zpair, pooled
GEN_LAYERS = [
    ("3a", 128, 256, 8, 28, 28, 14, False, False),
    ("3b", 256, 256, 8, 28, 28, 14, False, True),
    ("4a", 256, 512, 4, 14, 14, 14, True, False),
    ("4b", 512, 512, 4, 14, 14, 14, True, True),
    ("5a", 512, 512, 2, 7, 7, 7, True, False),
    ("5b", 512, 512, 2, 7, 7, 7, True, None),  # None -> raw stage (special pool)
]
PIPE_AR = ("3a", "3b", "4a")  # per-M-chunk AllReduce pipelining
BIN_LAYERS = ("4b", "5a", "5b")  # DFT4 frequency-domain circulant

TAPS = [(kd, kh, kw) for kd in range(3) for kh in range(3) for kw in range(3)]


def circ_expand_np(c):
    c = np.asarray(c, np.float32)
    P, Q, b = c.shape[0], c.shape[1], c.shape[2]
    r = np.arange(b)
    idx = (r[:, None] - r[None, :]) % b
    w = c[:, :, idx]  # (P, Q, b, b, k, k, k)
    w = np.transpose(w, (0, 2, 1, 3, 4, 5, 6))
    return w.reshape(P * b, Q * b, *c.shape[3:])


PERM512 = np.array([(n % 128) * 4 + n // 128 for n in range(512)])


def bin_weights(c):
    # c (128, 128, 4, 3,3,3) -> [128q, 27, 6, 128p] bf16 stationary mats
    c = np.asarray(c, np.float64)
    C0 = (c[:, :, 0] + c[:, :, 1] + c[:, :, 2] + c[:, :, 3]) / 4.0
    C2 = (c[:, :, 0] - c[:, :, 1] + c[:, :, 2] - c[:, :, 3]) / 4.0
    C1r = (c[:, :, 0] - c[:, :, 2]) / 2.0
    C1i = (c[:, :, 3] - c[:, :, 1]) / 2.0
    mats = [C0, C2, C1r, -C1i, C1i, C1r]  # per-tap (p,q,3,3,3) each
    w = np.stack([m.transpose(1, 0, 2, 3, 4) for m in mats], axis=0)
    # w: (6, q, p, kd, kh, kw) -> (q, 27, 6, p)
    w = w.transpose(1, 3, 4, 5, 0, 2).reshape(128, 27, 6, 128)
    return np.ascontiguousarray(w, dtype=NPBF16)


def pack_w_generic(wd, Kch, Mch):
    # wd (Co, Ci, 3,3,3) -> [Mch, 128ci, Kch, 27, 128co] bf16
    Co, Ci = wd.shape[0], wd.shape[1]
    wt = wd.transpose(1, 2, 3, 4, 0)  # (Ci, kd,kh,kw, Co)
    wt = wt.reshape(Kch, 128, 27, Mch, 128)
    wt = wt.transpose(3, 1, 0, 2, 4)  # (m, ci, c, t, co)
    return np.ascontiguousarray(wt, dtype=NPBF16)


def host_prep(inputs):
    g = {k: np.asarray(v, np.float32) for k, v in inputs.items()}
    shared = {}
    # conv1
    w1 = g["conv1_w"]  # (64, 3, 3,3,3)
    shared["w1"] = np.ascontiguousarray(
        w1.transpose(1, 2, 3, 4, 0).reshape(81, 64), dtype=NPBF16)
    # conv2 (kd-packed + kd=2 kh-packed)
    w2 = circ_expand_np(g["c2"])  # (128, 64, 3,3,3)
    w2t = w2.transpose(2, 1, 3, 4, 0)  # (kd, ci, kh, kw, co)
    shared["w2a"] = np.ascontiguousarray(
        w2t[0:2].reshape(128, 9, 128), dtype=NPBF16)
    kd2 = w2t[2]  # (64ci, 3kh, 3kw, 128co)
    shared["w2b01"] = np.ascontiguousarray(
        kd2[:, 0:2].transpose(1, 0, 2, 3).reshape(128, 3, 128), dtype=NPBF16)
    shared["w2b2"] = np.ascontiguousarray(kd2[:, 2], dtype=NPBF16)  # (64,3,128)
    # generic dense layers (4a outputs permuted to s-major for the bin layers)
    for (name, Cin, Cout, *_rest) in GEN_LAYERS:
        if name in BIN_LAYERS:
            shared[f"wf{name}"] = bin_weights(g[f"c{name}"])
        else:
            wd = circ_expand_np(g[f"c{name}"])
            if name == "4a":
                wd = wd[PERM512]
            shared[f"w{name}"] = pack_w_generic(wd, Cin // 128, Cout // 128)
    # bn params
    def pk(v, parts):
        v = np.asarray(v, np.float32)
        mch = v.size // parts
        return np.ascontiguousarray(v.reshape(mch, parts).T)
    shared["gn1"] = pk(g["g1"], 64)
    shared["bn1"] = pk(g["b1"], 64)
    for name, c in [("2", 128), ("3a", 256), ("3b", 256), ("4a", 512),
                    ("4b", 512), ("5a", 512), ("5b", 512)]:
        gv, bv = g[f"g{name}"], g[f"b{name}"]
        if name in ("4a",) + BIN_LAYERS:
            gv, bv = gv[PERM512], bv[PERM512]
        shared[f"gn{name}"] = pk(gv, 128)
        shared[f"bn{name}"] = pk(bv, 128)
        assert np.all(gv >= 0), "pool/BN commute needs g >= 0"
    assert np.all(g["g1"] >= 0)
    # fc (fold /16 global-mean into weights); rows permuted to s-major
    fcw = (g["fc_w"].T / 16.0)[PERM512]  # (512, 101)
    shared["fcw"] = np.ascontiguousarray(
        fcw.reshape(4, 128, 101).transpose(1, 0, 2), dtype=NPBF16)
    shared["fcb"] = np.ascontiguousarray(g["fc_b"].reshape(101, 1))
    # per-core conv1 im2col
    x = g["x"]  # (8, 3, 16, 112, 112)
    x1_list = []
    for i in range(x.shape[0]):
        xp = np.zeros((3, 18, 114, 114), np.float32)
        xp[:, 1:17, 1:113, 1:113] = x[i]
        sw = np.lib.stride_tricks.sliding_window_view(xp, (3, 3, 3), axis=(1, 2, 3))
        b1 = sw.transpose(0, 4, 5, 6, 1, 2, 3).reshape(81, 16, 12544)
        x1_list.append(np.ascontiguousarray(b1, dtype=NPBF16))
    return shared, x1_list


def build_bass(n_cores, fake_cc=False):
    nc = bacc.Bacc("TRN2", target_bir_lowering=False, debug=False,
                   num_devices=n_cores)
    rg = [list(range(n_cores))]

    din = {}
    din["x1"] = nc.dram_tensor("x1", [81, 16, 12544], BF16, kind="ExternalInput")
    din["w1"] = nc.dram_tensor("w1", [81, 64], BF16, kind="ExternalInput")
    din["w2a"] = nc.dram_tensor("w2a", [128, 9, 128], BF16, kind="ExternalInput")
    din["w2b01"] = nc.dram_tensor("w2b01", [128, 3, 128], BF16,
                                  kind="ExternalInput")
    din["w2b2"] = nc.dram_tensor("w2b2", [64, 3, 128], BF16,
                                 kind="ExternalInput")
    for (name, Cin, Cout, *_r) in GEN_LAYERS:
        if name in BIN_LAYERS:
            din[f"wf{name}"] = nc.dram_tensor(
                f"wf{name}", [128, 27, 6, 128], BF16, kind="ExternalInput")
        else:
            din[f"w{name}"] = nc.dram_tensor(
                f"w{name}", [Cout // 128, 128, Cin // 128, 27, 128], BF16,
                kind="ExternalInput")
    din["gn1"] = nc.dram_tensor("gn1", [64, 1], F32, kind="ExternalInput")
    din["bn1"] = nc.dram_tensor("bn1", [64, 1], F32, kind="ExternalInput")
    for name, c in [("2", 128), ("3a", 256), ("3b", 256), ("4a", 512),
                    ("4b", 512), ("5a", 512), ("5b", 512)]:
        mch = c // 128
        din[f"gn{name}"] = nc.dram_tensor(f"gn{name}", [128, mch], F32,
                                          kind="ExternalInput")
        din[f"bn{name}"] = nc.dram_tensor(f"bn{name}", [128, mch], F32,
                                          kind="ExternalInput")
    din["fcw"] = nc.dram_tensor("fcw", [128, 4, 101], BF16, kind="ExternalInput")
    din["fcb"] = nc.dram_tensor("fcb", [101, 1], F32, kind="ExternalInput")
    logits = nc.dram_tensor("logits", [101, 1], F32, kind="ExternalOutput")
    stats_out = {}
    for name, parts, mch in [("1", 64, 1), ("2", 128, 1), ("3a", 128, 2),
                             ("3b", 128, 2), ("4a", 128, 4), ("4b", 128, 4),
                             ("5a", 128, 4), ("5b", 128, 4)]:
        stats_out[name] = nc.dram_tensor(f"st{name}", [parts, mch * 2], F32,
                                         kind="ExternalOutput")

    with tile.TileContext(nc) as tc:
        build_graph(tc, din, logits, stats_out, rg, fake_cc)
    nc.compile()
    return nc


def build_graph(tc, din, logits, stats_out, rg, fake_cc=False):
    nc = tc.nc
    ndev = len(rg[0])
    import contextlib
    ctx = contextlib.ExitStack()
    with ctx:
        singles = ctx.enter_context(tc.tile_pool(name="singles", bufs=1))
        small = ctx.enter_context(tc.tile_pool(name="small", bufs=2))
        statsp = ctx.enter_context(tc.tile_pool(name="statsp", bufs=1))
        psum = ctx.enter_context(tc.tile_pool(name="psum", bufs=3, space="PSUM"))
        psfc = ctx.enter_context(tc.tile_pool(name="psfc", bufs=1, space="PSUM"))
        pwp = ctx.enter_context(tc.tile_pool(name="pwp", bufs=3))
        ybfp = ctx.enter_context(tc.tile_pool(name="ybfp", bufs=3))
        stagep = ctx.enter_context(tc.tile_pool(name="stagep", bufs=1))
        ypoolp = ctx.enter_context(tc.tile_pool(name="ypoolp", bufs=1))
        arena = ctx.enter_context(tc.tile_pool(name="arena", bufs=1))
        dram = ctx.enter_context(tc.tile_pool(name="dram", bufs=1, space="DRAM"))

        eps_t = singles.tile([128, 1], F32, tag="eps")
        nc.vector.memset(eps_t[:], EPS)

        # persistent small params
        params = {}
        for name, parts in [("1", 64), ("2", 128), ("3a", 128), ("3b", 128),
                            ("4a", 128), ("4b", 128), ("5a", 128), ("5b", 128)]:
            mch = din[f"gn{name}"].shape[1]
            gt = singles.tile([parts, mch], F32, tag=f"g{name}")
            bt = singles.tile([parts, mch], F32, tag=f"b{name}")
            nc.sync.dma_start(gt[:], din[f"gn{name}"][:])
            nc.sync.dma_start(bt[:], din[f"bn{name}"][:])
            params[name] = (gt, bt)

        w1_sb = singles.tile([81, 64], BF16, tag="w1")
        nc.sync.dma_start(w1_sb[:], din["w1"][:])
        w2a_sb = singles.tile([128, 9, 128], BF16, tag="w2a")
        nc.sync.dma_start(w2a_sb[:], din["w2a"][:])
        w2b01_sb = singles.tile([128, 3, 128], BF16, tag="w2b01")
        nc.sync.dma_start(w2b01_sb[:], din["w2b01"][:])
        w2b2_sb = singles.tile([64, 3, 128], BF16, tag="w2b2")
        nc.sync.dma_start(w2b2_sb[:], din["w2b2"][:])
        fcw_sb = singles.tile([128, 4, 101], BF16, tag="fcw")
        nc.sync.dma_start(fcw_sb[:], din["fcw"][:])
        fcb_sb = singles.tile([101, 1], F32, tag="fcb")
        nc.sync.dma_start(fcb_sb[:], din["fcb"][:])

        y1_dram = dram.tile([64, 16, 3136], BF16, tag="y1d")

        def do_cc(tag, ccin, width, parts):
            ccout = dram.tile([parts, width], F32, tag=f"co{tag}",
                              addr_space="Shared")
            if fake_cc:
                nc.sync.dma_start(ccout[:], ccin[:])
            else:
                nc.gpsimd.collective_compute(
                    "AllReduce", ADD, replica_groups=rg,
                    ins=[ccin.opt()], outs=[ccout.opt()])
            return ccout

        def stats_payload(tag, stats_slice, parts, scale=None):
            """bn_aggr a stats slice -> (w*mean, w*E[x^2]) tile [parts, 2]."""
            mv = small.tile([parts, 2], F32, tag=f"mv{tag}")
            nc.vector.bn_aggr(mv[:], stats_slice)
            sq = small.tile([parts, 1], F32, tag=f"sk{tag}")
            nc.vector.tensor_mul(sq[:], mv[:, 0:1], mv[:, 0:1])
            cc = small.tile([parts, 2], F32, tag=f"cc{tag}")
            nc.vector.tensor_add(cc[:, 1:2], mv[:, 1:2], sq[:])
            if scale is not None:
                nc.vector.tensor_scalar_mul(cc[:, 0:1], mv[:, 0:1], scale)
                nc.vector.tensor_scalar_mul(cc[:, 1:2], cc[:, 1:2], scale)
            else:
                nc.vector.tensor_copy(cc[:, 0:1], mv[:, 0:1])
            return cc

        def chain_core(utag, mE, parts, mch, gt_ap, bt_ap):
            """mE [parts, mch, 2] = (E[x], E[x^2]) -> BN scale s, shift t."""
            sq = small.tile([parts, mch], F32, tag=f"sq{utag}")
            nc.vector.tensor_mul(sq[:], mE[:, :, 0], mE[:, :, 0])
            var = small.tile([parts, mch], F32, tag=f"vr{utag}")
            nc.vector.tensor_sub(var[:], mE[:, :, 1], sq[:])
            sd = small.tile([parts, mch], F32, tag=f"sd{utag}")
            nc.scalar.activation(sd[:], var[:], SQRT, bias=eps_t[:parts])
            inv = small.tile([parts, mch], F32, tag=f"iv{utag}")
            nc.vector.reciprocal(inv[:], sd[:])
            s_t = small.tile([parts, mch], F32, tag=f"ss{utag}")
            nc.vector.tensor_mul(s_t[:], inv[:], gt_ap)
            u = small.tile([parts, mch], F32, tag=f"uu{utag}")
            nc.vector.tensor_mul(u[:], mE[:, :, 0], s_t[:])
            t_t = small.tile([parts, mch], F32, tag=f"tt{utag}")
            nc.vector.tensor_sub(t_t[:], bt_ap, u[:])
            return s_t, t_t

        def zero_borders(P, mch, Dp, Hp, Wp):
            for c in range(mch):
                nc.vector.memset(P[:, c, 0], 0.0)
                nc.vector.memset(P[:, c, Dp - 1], 0.0)
                nc.vector.memset(P[:, c, 1:Dp - 1, 0, :], 0.0)
                nc.vector.memset(P[:, c, 1:Dp - 1, Hp - 1, :], 0.0)
                nc.vector.memset(P[:, c, 1:Dp - 1, 1:Hp - 1, 0:1], 0.0)
                nc.vector.memset(P[:, c, 1:Dp - 1, 1:Hp - 1, Wp - 1:Wp], 0.0)

        # ---------------- conv1 ----------------
        # 224 row-tile pairs; j=0 -> PSUM partitions 0-63, j=1 -> 64-127
        # (PE column tiling), so stats/pool run on 128 partitions.
        stats1 = statsp.tile([128, 1, 224, 6], F32, tag="stats")
        cc1_outs = []

        def conv1_ar(tag, lo, hi):
            cs = stats_payload(tag, stats1[:, 0, lo:hi], 128,
                               scale=(hi - lo) / 224.0)
            ccin = dram.tile([64, 2, 2], F32, tag=f"ci{tag}")
            nc.sync.dma_start(ccin[:, :, 0], cs[0:64, :])
            nc.sync.dma_start(ccin[:, :, 1], cs[64:128, :])
            cc1_outs.append(do_cc(tag, ccin, 4, 64))

        with tc.tile_pool(name="x1p", bufs=2) as x1p, nc.named_scope("conv1"):
            for z in range(16):
                for half in range(2):
                    slab = x1p.tile([81, 6272], BF16, tag="slab")
                    for pc in range(7):
                        nc.sync.dma_start(
                            slab[:, pc * 896:(pc + 1) * 896],
                            din["x1"][:, z, half * 6272 + pc * 896:
                                      half * 6272 + (pc + 1) * 896])
                    for t2 in range(7):
                        pst = psum.tile([128, 2, 512], F32, tag="ps", name="ps")
                        for j in range(2):
                            t = 2 * t2 + j
                            nc.tensor.matmul(pst[64 * j:64 * j + 64, 0, :448],
                                             w1_sb[:],
                                             slab[:, t * 448:(t + 1) * 448],
                                             start=True, stop=True)
                        ti = z * 14 + half * 7 + t2
                        nc.vector.bn_stats(stats1[:, 0, ti], pst[:, 0, :448])
                        ybf = ybfp.tile([128, 448], BF16, tag="ybf", name="ybf")
                        nc.scalar.activation(ybf[:], pst[:, 0, :448], COPY)
                        v = ybf[:].rearrange("p (a b) -> p a b", a=4)
                        pw = pwp.tile([128, 4, 56], BF16, tag="pw", name="pw")
                        nc.vector.tensor_max(pw[:], v[:, :, 0::2], v[:, :, 1::2])
                        zp = pwp.tile([128, 2, 56], BF16, tag="zp", name="zp")
                        nc.vector.tensor_max(zp[:], pw[:, 0::2, :],
                                             pw[:, 1::2, :])
                        ro = (28 * half + 4 * t2) * 56
                        nc.sync.dma_start(
                            y1_dram[:, z, ro:ro + 112],
                            zp[0:64].rearrange("p a b -> p (a b)"))
                        nc.sync.dma_start(
                            y1_dram[:, z, ro + 112:ro + 224],
                            zp[64:128].rearrange("p a b -> p (a b)"))
                if z == 14:
                    with nc.named_scope("ar1a"):
                        conv1_ar("1a", 0, 210)
        with nc.named_scope("ar1b"):
            conv1_ar("1b", 210, 224)
            ar1a = small.tile([64, 2, 2], F32, tag="ar1a")
            nc.sync.dma_start(ar1a[:], cc1_outs[0][:])
            ar1b = small.tile([64, 2, 2], F32, tag="ar1b")
            nc.sync.dma_start(ar1b[:], cc1_outs[1][:])
            tot3 = small.tile([64, 2, 2], F32, tag="tot3")
            nc.vector.tensor_add(tot3[:], ar1a[:], ar1b[:])
            tot1 = small.tile([64, 1, 2], F32, tag="tot1")
            nc.vector.tensor_add(tot1[:, 0], tot3[:, :, 0], tot3[:, :, 1])
            nc.sync.dma_start(stats_out["1"][:], tot1[:, 0])
            mE1 = small.tile([64, 1, 2], F32, tag="mE1")
            nc.vector.tensor_scalar_mul(mE1[:], tot1[:], 1.0 / (2 * ndev))
            g1t, b1t = params["1"]
            s1, t1 = chain_core("1", mE1, 64, 1, g1t[:], b1t[:])

        # ---------------- conv2 ----------------
        stats2 = statsp.tile([128, 1, 128, 6], F32, tag="stats")
        S2 = stagep.tile([128, 1, 16, 28, 28], BF16, tag="stage")
        Y2p = ypoolp.tile([128, 1, 8, 28, 28], BF16, tag="ypool2")
        P3in = arena.tile([128, 1, 10, 30, 30], BF16, tag="pinA")
        zero_borders(P3in, 1, 10, 30, 30)
        cc2_outs = []
        with tc.tile_pool(name="plp", bufs=2) as plp, \
             tc.tile_pool(name="b2p", bufs=2) as b2p, \
             tc.tile_pool(name="c2p", bufs=2) as c2p, \
             nc.named_scope("conv2"):

            def fetch_plane(pidx):
                pl = plp.tile([64, 3136], BF16, tag="pl")
                nc.sync.dma_start(pl[:], y1_dram[:, pidx - 1, :])
                return pl

            def build_plane(dst64, pidx):
                if pidx == 0 or pidx == 17:
                    nc.vector.memset(dst64[:], 0.0)
                    return
                pl = fetch_plane(pidx)
                nc.vector.memset(dst64[:, 0, :], 0.0)
                nc.vector.memset(dst64[:, 57, :], 0.0)
                nc.vector.memset(dst64[:, 1:57, 0:1], 0.0)
                nc.vector.memset(dst64[:, 1:57, 57:58], 0.0)
                nc.scalar.activation(
                    dst64[:, 1:57, 1:57],
                    pl[:].rearrange("p (a b) -> p a b", a=56),
                    RELU, bias=t1[:, 0:1], scale=s1[:, 0:1])
                return pl

            def build_plane_sh(dst64, pidx, pl):
                # dst64[y] = padded_plane[y + 1]  (kh=1 shifted copy)
                if pidx == 0 or pidx == 17:
                    nc.vector.memset(dst64[:], 0.0)
                    return
                nc.vector.memset(dst64[:, 56, :], 0.0)
                nc.vector.memset(dst64[:, 57, :], 0.0)
                nc.vector.memset(dst64[:, 0:56, 0:1], 0.0)
                nc.vector.memset(dst64[:, 0:56, 57:58], 0.0)
                nc.scalar.activation(
                    dst64[:, 0:56, 1:57],
                    pl[:].rearrange("p (a b) -> p a b", a=56),
                    RELU, bias=t1[:, 0:1], scale=s1[:, 0:1])

            for z in range(16):
                B2 = b2p.tile([128, 58, 58], BF16, tag="b2")
                build_plane(B2[0:64], z)
                build_plane(B2[64:128], z + 1)
                C2 = c2p.tile([128, 58, 58], BF16, tag="c2")
                plc = build_plane(C2[0:64], z + 2)
                build_plane_sh(C2[64:128], z + 2, plc)
                for p2 in range(4):
                    pst = psum.tile([128, 2, 512], F32, tag="ps", name="ps")
                    for k9 in range(9):
                        kh, kw = k9 // 3, k9 % 3
                        for j in range(2):
                            y0 = 14 * p2 + 7 * j + kh
                            nc.tensor.matmul(pst[:, j, :392], w2a_sb[:, k9, :],
                                             B2[:, y0:y0 + 7, kw:kw + 56],
                                             start=(k9 == 0), stop=False)
                    for kw in range(3):
                        for j in range(2):
                            y0 = 14 * p2 + 7 * j
                            nc.tensor.matmul(pst[:, j, :392],
                                             w2b01_sb[:, kw, :],
                                             C2[:, y0:y0 + 7, kw:kw + 56],
                                             start=False, stop=False)
                    for kw in range(3):
                        for j in range(2):
                            y0 = 14 * p2 + 7 * j + 2
                            nc.tensor.matmul(pst[:, j, :392],
                                             w2b2_sb[:, kw, :],
                                             C2[0:64, y0:y0 + 7, kw:kw + 56],
                                             start=False, stop=(kw == 2))
                    ti = z * 8 + 2 * p2
                    nc.vector.bn_stats(stats2[:, 0, ti], pst[:, 0, :392])
                    nc.vector.bn_stats(stats2[:, 0, ti + 1], pst[:, 1, :392])
                    ybf = ybfp.tile([128, 2, 392], BF16, tag="ybf2", name="ybf")
                    nc.scalar.activation(ybf[:], pst[:, :2, :392], COPY)
                    v = ybf[:].rearrange("p g (a b) -> p (g a) b", a=7)
                    pw = pwp.tile([128, 14, 28], BF16, tag="pw2", name="pw")
                    nc.vector.tensor_max(pw[:], v[:, :, 0::2], v[:, :, 1::2])
                    nc.vector.tensor_max(S2[:, 0, z, 7 * p2:7 * p2 + 7, :],
                                         pw[:, 0::2, :], pw[:, 1::2, :])
                if z % 2 == 1:
                    nc.vector.tensor_max(Y2p[:, 0, z // 2], S2[:, 0, z - 1],
                                         S2[:, 0, z])
                if z == 14:
                    with nc.named_scope("ar2a"):
                        cs = stats_payload("2a", stats2[:, 0, 0:120], 128,
                                           scale=120.0 / 128.0)
                        ccin = dram.tile([128, 2], F32, tag="ci2a")
                        nc.sync.dma_start(ccin[:], cs[:])
                        cc2_outs.append(do_cc("2a", ccin, 2, 128))
        with nc.named_scope("ar2b"):
            cs = stats_payload("2b", stats2[:, 0, 120:128], 128,
                               scale=8.0 / 128.0)
            ccin = dram.tile([128, 2], F32, tag="ci2b")
            nc.sync.dma_start(ccin[:], cs[:])
            cc2_outs.append(do_cc("2b", ccin, 2, 128))
            ar2a = small.tile([128, 2], F32, tag="ar2a")
            nc.sync.dma_start(ar2a[:], cc2_outs[0][:])
            ar2b = small.tile([128, 2], F32, tag="ar2b")
            nc.sync.dma_start(ar2b[:], cc2_outs[1][:])
            tot2 = small.tile([128, 1, 2], F32, tag="tot2")
            nc.vector.tensor_add(tot2[:, 0], ar2a[:], ar2b[:])
            nc.sync.dma_start(stats_out["2"][:], tot2[:, 0])
            mE2 = small.tile([128, 1, 2], F32, tag="mE2")
            nc.vector.tensor_scalar_mul(mE2[:], tot2[:], 1.0 / ndev)
            g2t, b2t = params["2"]
            s2, t2 = chain_core("2", mE2, 128, 1, g2t[:], b2t[:])
        nc.scalar.activation(P3in[:, 0, 1:9, 1:29, 1:29], Y2p[:, 0], RELU,
                             bias=t2[:, 0:1], scale=s2[:, 0:1])

        # ---------------- generic conv layers ----------------
        if True:
            Pin = P3in
            wp_cm = tc.tile_pool(name="wp", bufs=2)
            wp = wp_cm.__enter__()
            wfp_cm = None
            for li, (name, Cin, Cout, D, H, W, R, zpair, pooled) in \
                    enumerate(GEN_LAYERS):
                if li == 3:
                    wp_cm.__exit__(None, None, None)
                    wfp_cm = tc.tile_pool(name="wfp", bufs=1)
                    wfp = wfp_cm.__enter__()
                Kch, Mch = Cin // 128, Cout // 128
                ntz = D // 2 if zpair else D
                zcnt = 2 if zpair else 1
                ytiles = H // R
                N = zcnt * R * W
                T = ntz * ytiles
                H2, W2, D2 = H // 2, W // 2, D // 2
                per_m = name in PIPE_AR
                gt, bt = params[name]
                stats_t = statsp.tile([128, Mch, T, 6], F32, tag="stats")
                if pooled is False or pooled is None:
                    stage = stagep.tile([128, Mch, D, H, W], BF16, tag="stage")
                else:
                    stage = stagep.tile([128, Mch, D, H2, W2], BF16, tag="stage")
                stage_flat = stage[:].rearrange("p m d h w -> p (m d h w)")
                if name != "5b":
                    if pooled:
                        nD, nH, nW = D2, H2, W2
                    else:
                        nD, nH, nW = D, H, W
                    Pnext = arena.tile([128, Mch, nD + 2, nH + 2, nW + 2],
                                       BF16, tag="pinB" if name in ("3a", "4a") else "pinA")
                    zero_borders(Pnext, Mch, nD + 2, nH + 2, nW + 2)
                else:
                    ph5 = small.tile([128, 4, 4, 4], BF16, tag="ph5")
                tiles = [(2 * tz if zpair else tz, ty * R)
                         for tz in range(ntz) for ty in range(ytiles)]
                groups = [tiles[i:i + 2] for i in range(0, len(tiles), 2)]
                sm_list = [None] * Mch
                scope = nc.named_scope(f"conv{name}")
                scope.__enter__()
                if name in BIN_LAYERS:
                    Dp, Hp, Wp = D + 2, H + 2, W + 2
                    PF = arena.tile([128, 4, Dp, Hp, Wp], BF16, tag="pf")
                    V = Dp * Hp * Wp
                    pin_f = [Pin[:, c2].rearrange("p d h w -> p (d h w)")
                             for c2 in range(4)]
                    pf_f = [PF[:, c2].rearrange("p d h w -> p (d h w)")
                            for c2 in range(4)]
                    s02 = ybfp.tile([128, V], BF16, tag="s02", bufs=1)
                    s13 = ybfp.tile([128, V], BF16, tag="s13", bufs=1)
                    nc.vector.tensor_add(s02[:], pin_f[0], pin_f[2])
                    nc.vector.tensor_add(s13[:], pin_f[1], pin_f[3])
                    nc.vector.tensor_add(pf_f[0], s02[:], s13[:])
                    nc.vector.tensor_sub(pf_f[1], s02[:], s13[:])
                    nc.vector.tensor_sub(pf_f[2], pin_f[0], pin_f[2])
                    nc.vector.tensor_sub(pf_f[3], pin_f[3], pin_f[1])
                    for gi, (z0, y0) in enumerate(tiles):
                        pstA = psum.tile([128, 2, 512], F32, tag="ps",
                                         name="ps")
                        pstB = psum.tile([128, 2, 512], F32, tag="ps",
                                         name="ps")
                        for i6, (kd, kh, kw) in enumerate(TAPS):
                            if i6 % 9 == 0:
                                wmf = wfp.tile([128, 9, 6, 128], BF16,
                                               tag="wf", bufs=2)
                                nc.sync.dma_start(
                                    wmf[:],
                                    din[f"wf{name}"][:, i6:i6 + 9])
                            st0, sp0 = (i6 == 0), (i6 == 26)
                            rh = [PF[:, c2, z0 + kd:z0 + kd + 2,
                                     y0 + kh:y0 + kh + R, kw:kw + W]
                                  for c2 in range(4)]
                            wt6 = wmf[:, i6 % 9]
                            nc.tensor.matmul(pstA[:, 0, :N], wt6[:, 0, :],
                                             rh[0], start=st0, stop=sp0)
                            nc.tensor.matmul(pstA[:, 1, :N], wt6[:, 1, :],
                                             rh[1], start=st0, stop=sp0)
                            nc.tensor.matmul(pstB[:, 0, :N], wt6[:, 2, :],
                                             rh[2], start=st0, stop=False)
                            nc.tensor.matmul(pstB[:, 0, :N], wt6[:, 3, :],
                                             rh[3], start=False, stop=sp0)
                            nc.tensor.matmul(pstB[:, 1, :N], wt6[:, 4, :],
                                             rh[2], start=st0, stop=False)
                            nc.tensor.matmul(pstB[:, 1, :N], wt6[:, 5, :],
                                             rh[3], start=False, stop=sp0)
                        y0c = ybfp.tile([128, 512], F32, tag="y0c",
                                        name="y0c", bufs=2)
                        nc.scalar.activation(y0c[:, :N], pstA[:, 0, :N], COPY)
                        at = ybfp.tile([128, 512], F32, tag="at", name="at",
                                       bufs=1)
                        bt2 = ybfp.tile([128, 512], F32, tag="bt2",
                                        name="bt2", bufs=1)
                        nc.vector.tensor_add(at[:, :N], y0c[:, :N],
                                             pstA[:, 1, :N])
                        nc.vector.tensor_sub(bt2[:, :N], y0c[:, :N],
                                             pstA[:, 1, :N])
                        yi = ybfp.tile([128, 4, 512], BF16, tag="yi",
                                       name="yi", bufs=1)
                        nc.vector.tensor_add(yi[:, 0, :N], at[:, :N],
                                             pstB[:, 0, :N])
                        nc.vector.tensor_sub(yi[:, 2, :N], at[:, :N],
                                             pstB[:, 0, :N])
                        nc.vector.tensor_sub(yi[:, 1, :N], bt2[:, :N],
                                             pstB[:, 1, :N])
                        nc.vector.tensor_add(yi[:, 3, :N], bt2[:, :N],
                                             pstB[:, 1, :N])
                        for r2 in range(4):
                            nc.vector.bn_stats(stats_t[:, r2, gi],
                                               yi[:, r2, :N])
                        if pooled is True:
                            for r2 in range(4):
                                v = yi[:, r2, :N].rearrange(
                                    "p (zz y x) -> p zz y x", zz=2, y=R)
                                pw = pwp.tile([128, 2, R, W2], BF16,
                                              tag="pwb", name="pw")
                                nc.vector.tensor_max(pw[:], v[:, :, :, 0::2],
                                                     v[:, :, :, 1::2])
                                nc.vector.tensor_max(
                                    stage[:, r2, z0:z0 + 2, :, :],
                                    pw[:, :, 0::2, :], pw[:, :, 1::2, :])
                        else:
                            for r2 in range(4):
                                nc.scalar.activation(
                                    stage[:, r2, z0:z0 + 2, :, :],
                                    yi[:, r2, :N].rearrange(
                                        "p (zz y x) -> p zz y x", zz=2, y=R),
                                    COPY)
                    if name == "5b":
                        for m in range(4):
                            pd = small.tile([128, 7, 7], BF16, tag=f"pd5{m}")
                            nc.vector.tensor_max(pd[:], stage[:, m, 0],
                                                 stage[:, m, 1])
                            pw5 = small.tile([128, 7, 4], BF16, tag=f"pw5{m}")
                            nc.vector.tensor_copy(pw5[:, :, 0:1],
                                                  pd[:, :, 0:1])
                            nc.vector.tensor_max(pw5[:, :, 1:4],
                                                 pd[:, :, 1::2],
                                                 pd[:, :, 2::2])
                            nc.vector.tensor_copy(ph5[:, m, 0:1, :],
                                                  pw5[:, 0:1, :])
                            nc.vector.tensor_max(ph5[:, m, 1:4, :],
                                                 pw5[:, 1::2, :],
                                                 pw5[:, 2::2, :])
                else:
                 for m in range(Mch):
                    wm = wp.tile([128, Kch, 27, 128], BF16, tag="w")
                    for c in range(Kch):
                        nc.sync.dma_start(wm[:, c], din[f"w{name}"][m, :, c])
                    ti = 0
                    for grp in groups:
                        G = len(grp)
                        pst = psum.tile([128, 2, 512], F32, tag="ps",
                                        name="ps")
                        nmm = Kch * 27
                        i = 0
                        for c in range(Kch):
                            for (kd, kh, kw) in TAPS:
                                for j, (z0, y0) in enumerate(grp):
                                    rhs = Pin[:, c, z0 + kd:z0 + kd + zcnt,
                                              y0 + kh:y0 + kh + R,
                                              kw:kw + W]
                                    nc.tensor.matmul(
                                        pst[:, j, :N],
                                        wm[:, c, kd * 9 + kh * 3 + kw, :],
                                        rhs, start=(i == 0), stop=(i == nmm - 1))
                                i += 1
                        z0, y0 = grp[0]
                        for j in range(G):
                            nc.vector.bn_stats(stats_t[:, m, ti + j],
                                               pst[:, j, :N])
                        if pooled is False or pooled is None:
                            off = (m * D + z0) * H * W + y0 * W
                            dst = stage_flat[:, off:off + G * N]
                            nc.scalar.activation(
                                dst.rearrange("p (g n) -> p g n", g=G),
                                pst[:, :G, :N], COPY)
                        else:
                            ybf = ybfp.tile([128, 1024], BF16, tag="ybf",
                                            name="ybf")
                            nc.scalar.activation(
                                ybf[:, :G * N].rearrange(
                                    "p (g n) -> p g n", g=G),
                                pst[:, :G, :N], COPY)
                            nz = G * zcnt if zpair else 1
                            nr = R if zpair else G * R
                            v = ybf[:, :G * N].rearrange(
                                "p (z y x) -> p z y x", z=nz, y=nr)
                            pw = pwp.tile([128, nz, nr, W2], BF16,
                                          tag="pwg", name="pw")
                            nc.vector.tensor_max(pw[:], v[:, :, :, 0::2],
                                                 v[:, :, :, 1::2])
                            nc.vector.tensor_max(
                                stage[:, m, z0:z0 + nz,
                                      y0 // 2:y0 // 2 + nr // 2, :],
                                pw[:, :, 0::2, :], pw[:, :, 1::2, :])
                        ti += G
                    # ---- per-m tail ----
                    if name == "5b":
                        # raw special pool (independent of BN stats)
                        pd = small.tile([128, 7, 7], BF16, tag=f"pd5{m}")
                        nc.vector.tensor_max(pd[:], stage[:, m, 0],
                                             stage[:, m, 1])
                        pw5 = small.tile([128, 7, 4], BF16, tag=f"pw5{m}")
                        nc.vector.tensor_copy(pw5[:, :, 0:1], pd[:, :, 0:1])
                        nc.vector.tensor_max(pw5[:, :, 1:4],
                                             pd[:, :, 1::2], pd[:, :, 2::2])
                        nc.vector.tensor_copy(ph5[:, m, 0:1, :],
                                              pw5[:, 0:1, :])
                        nc.vector.tensor_max(ph5[:, m, 1:4, :],
                                             pw5[:, 1::2, :], pw5[:, 2::2, :])
                    elif per_m:
                        with nc.named_scope(f"ar{name}{m}"):
                            cs = stats_payload(f"{name}{m}", stats_t[:, m], 128)
                            ccin = dram.tile([128, 2], F32,
                                             tag=f"ci{name}{m}")
                            nc.sync.dma_start(ccin[:], cs[:])
                            cco = do_cc(f"{name}{m}", ccin, 2, 128)
                            ar_m = small.tile([128, 1, 2], F32,
                                              tag=f"ar{name}{m}")
                            nc.sync.dma_start(ar_m[:, 0], cco[:])
                            nc.sync.dma_start(stats_out[name][:, 2 * m:2 * m + 2],
                                              ar_m[:, 0])
                            mE_m = small.tile([128, 1, 2], F32,
                                              tag=f"me{name}{m}")
                            nc.vector.tensor_scalar_mul(mE_m[:], ar_m[:],
                                                        1.0 / ndev)
                            s_m, t_m = chain_core(f"{name}{m}", mE_m, 128, 1,
                                                  gt[:, m:m + 1], bt[:, m:m + 1])
                        if pooled:
                            src = ypoolp.tile([128, D2, H2, W2], BF16,
                                              tag=f"yp{name}{m}")
                            nc.vector.tensor_max(src[:], stage[:, m, 0::2],
                                                 stage[:, m, 1::2])
                            src_ap = src[:]
                        else:
                            src_ap = stage[:, m]
                        nc.scalar.activation(
                            Pnext[:, m, 1:1 + nD, 1:1 + nH, 1:1 + nW],
                            src_ap, RELU,
                            bias=t_m[:, 0:1], scale=s_m[:, 0:1])
                scope.__exit__(None, None, None)

                if not per_m:
                    # single whole-layer AllReduce (5a / 5b)
                    with nc.named_scope(f"ar{name}"):
                        cs_all = small.tile([128, Mch, 2], F32,
                                            tag=f"cs{name}")
                        for m in range(Mch):
                            mv = small.tile([128, 2], F32, tag=f"mv{name}{m}")
                            nc.vector.bn_aggr(mv[:], stats_t[:, m])
                            sq = small.tile([128, 1], F32, tag=f"sk{name}{m}")
                            nc.vector.tensor_mul(sq[:], mv[:, 0:1], mv[:, 0:1])
                            nc.vector.tensor_add(cs_all[:, m, 1:2],
                                                 mv[:, 1:2], sq[:])
                            nc.vector.tensor_copy(cs_all[:, m, 0:1],
                                                  mv[:, 0:1])
                        ccin = dram.tile([128, Mch * 2], F32, tag=f"ci{name}")
                        nc.sync.dma_start(
                            ccin[:], cs_all[:].rearrange("p m two -> p (m two)"))
                        cco = do_cc(name, ccin, Mch * 2, 128)
                        ar = small.tile([128, Mch, 2], F32, tag=f"ar{name}")
                        nc.sync.dma_start(
                            ar[:].rearrange("p m two -> p (m two)"), cco[:])
                        nc.sync.dma_start(
                            stats_out[name][:],
                            ar[:].rearrange("p m two -> p (m two)"))
                        mE = small.tile([128, Mch, 2], F32, tag=f"me{name}")
                        nc.vector.tensor_scalar_mul(mE[:], ar[:], 1.0 / ndev)
                        s_t, t_t = chain_core(name, mE, 128, Mch, gt[:], bt[:])

                    if name == "5b":
                        # BN+ReLU -> Z, global mean (1/16 in fcw), FC
                        Z = small.tile([128, 4, 16], BF16, tag="z5")
                        for m in range(4):
                            nc.scalar.activation(
                                Z[:, m, :],
                                ph5[:, m].rearrange("p a b -> p (a b)"),
                                RELU, bias=t_t[:, m:m + 1],
                                scale=s_t[:, m:m + 1])
                        feat = small.tile([128, 4], F32, tag="feat")
                        nc.vector.tensor_reduce(feat[:], Z[:],
                                                axis=mybir.AxisListType.X,
                                                op=ADD)
                        fcin = small.tile([128, 4], BF16, tag="fcin")
                        nc.vector.tensor_copy(fcin[:], feat[:])
                        psf = psfc.tile([101, 1], F32, tag="psfc")
                        for c in range(4):
                            nc.tensor.matmul(psf[:], fcw_sb[:, c, :],
                                             fcin[:, c:c + 1],
                                             start=(c == 0), stop=(c == 3))
                        out_sb = small.tile([101, 1], F32, tag="outsb")
                        nc.scalar.activation(out_sb[:], psf[:], IDENT,
                                             bias=fcb_sb[:])
                        nc.sync.dma_start(logits[:], out_sb[:])
                        break

                    # applies (4b pooled, 5a plain)
                    for m in range(Mch):
                        if pooled:
                            src = ypoolp.tile([128, D2, H2, W2], BF16,
                                              tag=f"yp{name}{m}")
                            nc.vector.tensor_max(src[:], stage[:, m, 0::2],
                                                 stage[:, m, 1::2])
                            src_ap = src[:]
                        else:
                            src_ap = stage[:, m]
                        nc.scalar.activation(
                            Pnext[:, m, 1:1 + nD, 1:1 + nH, 1:1 + nW],
                            src_ap, RELU,
                            bias=t_t[:, m:m + 1], scale=s_t[:, m:m + 1])
                Pin = Pnext
            if wfp_cm is not None:
                wfp_cm.__exit__(None, None, None)
            else:
                wp_cm.__exit__(None, None, None)


_STATE = {}


def _get_nc(n_cores=N_CORES):
    key = f"nc{n_cores}"
    if key not in _STATE:
        _STATE[key] = build_bass(n_cores)
    return _STATE[key]


def kernel(**inputs):
    nc = _get_nc()
    shared, x1_list = host_prep(inputs)
    in_maps = []
    for i in range(N_CORES):
        m = dict(shared)
        m["x1"] = x1_list[i]
        in_maps.append(m)
    res = run_bass_kernel_spmd(nc, in_maps, core_ids=list(range(N_CORES)))
    out = np.stack([res.results[i]["logits"].reshape(101)
                    for i in range(N_CORES)]).astype(np.float32)
    return out
